# revision 72
# baseline (speedup 1.0000x reference)
"""Trainium2 Bass kernel for nn_GBSTokenizer.

Math: the reference's route softmax is over a size-1 axis, so the route
probabilities are exactly 1.0 and the L x L calibration matmul collapses to a
scalar ~1 (verified |s-1| < 6e-8, output deviation < 6e-7 absmax-relative).
The computation therefore reduces to

    out = poolsum(conv1d(X, conv_w) + conv_b) @ wd + bd

where poolsum(z)[l] = z[l] + mean2[l//2] + mean3[l//3] + mean4[l//4]
(multi-scale block means, blocks of size 2/3/4). Since everything between
the conv and the final projection is linear, wd is folded into the conv
weights on the host:  Wk_eff = conv_w[:,:,k].T @ wd, so the device computes

    Z[dout, l] = sum_k  Wk_eff[k].T @ XT[:, l+k-1]
    S = poolsum_over_l(Z) + const                          (vector ops)
    out[l, :]  = S[:, l]                                   (host transpose)

with const = 4*(conv_b @ wd) + bd added as a per-partition bias at PSUM
eviction (divided by 4 because poolsum multiplies constants by exactly 4).

Matmul precision: fp8(e4m3) DoubleRow with hi/lo error compensation.
Both W and X are split host-side into fp8 pairs at a shared power-of-2
scale (W*64 = Wh + Wl, X*16 = Xh + Xl, residuals stored at the same
scale so everything accumulates in one PSUM group):

    Z ~= (Xh@Wh + Xh@Wl + Xl@Wh) / 1024

Each DoubleRow matmul contracts 2 x 128 at 0.5 cycles/output column.
Corrections are asymmetric: 10 k-tiles get both slots, 8 k-tiles keep
only the X-side slot (Xl@Wh, paired like the main pass on the lo
plane), for 11.5 cyc/col vs 18 for bf16.

Chunks 2-3 of dtiles 0-3 instead use Winograd F(2,3) (see the W_CHUNKS
comment below): 12 k-tiles per output column with FULL corrections on
every tile = 9 cyc/col at ~4x lower quantization error than the direct
path.  More Winograd units don't pay: the A-transform adds DVE/Pool work
and those engines' effective time (busy + ~200ns/op semaphore hops)
already sits just under the PE window; past ~80% utilization the backlog
drains as post-PE tail.  Measured on HW: 1.892e-2 absmax-rel / 1.508e-2
rms-rel (gate 2e-2; absmax comes from the direct-path units).

Pooling combine per 408-col chunk (divisible by 12 = lcm(2,3,4), so all
blocks are chunk-local), with sum2[j] = pairsum, sum3[t] = triplesum:
    W [i] = 3*sum2[2i] +   sum2[2i+1]
    W'[i] =   sum2[2i] + 3*sum2[2i+1]
    S[4i+{0,1}] = Z + 0.25*W[i];  S[4i+{2,3}] = Z + 0.25*W'[i]
    S[3t+p]    += (1/3)*sum3[t]
The final combine writes bf16 (halves the output DMA).

Sharding: data-parallel over batch N=8, one sample per NeuronCore, params
replicated. All compute layouts keep feature dim on partitions and sequence
dim on the free axis (transposed), so the host transposes X in and out.
"""

import numpy as np
import ml_dtypes

# Problem shape (hardcoded per harness contract).
N_SAMPLES = 8
L = 2040
D = 768
NT = D // 128          # 6 partition tiles over features
NCH = 5                # l-chunks per psum pass
CHW = L // NCH         # 408 columns per chunk (<=512 fp32 = 1 PSUM bank),
                       # divisible by 12 so pooling is chunk-local
HALF = CHW             # full-chunk DoubleRow matmuls (moving free 816;
                       # the 512 limit is not enforced by walrus — validated
                       # numerically on HW)
NH = 1                 # matmul column-splits per chunk
NKT = 3 * NT           # 18 k-tiles (di-major: idx = di*3 + k)
NCOR = 10              # k-tiles with FULL hi/lo corrections; tiles
                       # NCOR..17 get the X-side correction slot only.
                       # HW measures 1.892e-2 absmax-rel vs the 2e-2 gate
                       # (above the host model's 1.778e-2 — the PE appears
                       # to flush fp8 subnormals on slot-0 operands, which
                       # weakens half-corrections; scaled-copy fixes cost
                       # more early DMA than they save in PE time)
N_CORES = 8

SW = 64.0              # weight fp8 scale
SX = 16.0              # activation fp8 scale
BF16 = ml_dtypes.bfloat16
E4M3 = ml_dtypes.float8_e4m3

# --- Winograd F(2,3) path (chunks 2..3) -----------------------------------
# For pair j (output cols 2j, 2j+1):
#   m1 = G0^T (x_{2j-1} - x_{2j+1});  m2 = Ga^T (x_{2j} + x_{2j+1})
#   m3 = Gb^T (x_{2j+1} - x_{2j});    m4'= G2n^T(x_{2j} - x_{2j+2})
# with Ga = (G0+G1+G2)/2, Gb = (G0-G1+G2)/2, G2n = -G2 folded on the host.
#   z_even = m1 + m2 + m3;  z_odd = m2 - m3 + m4'  (classic A-transform)
# 12 k-tiles of contraction per output column instead of 18, with FULL
# hi/lo corrections on every tile (0.75 cyc/ktile): 9 cyc/col vs the
# direct path's 11.5, at LOWER quantization error (~0.5% vs 1.9%, since
# nothing is half-corrected).  The 2x-finer W-unit error also absorbs the
# bf16 intermediates of the reconstruction.
W_CHUNKS = (2, 3)      # chunk indices computed via Winograd (all dtiles)
WBASE = W_CHUNKS[0] * CHW        # first Winograd output column
WBASE_PAIR = WBASE // 2          # first Winograd pair
NWP = len(W_CHUNKS) * CHW // 2   # Winograd pairs total
XTSTR = 4 * NT * 2               # bytes per pair-column in xt (s x di x pl)
SX2 = 32.0             # Winograd activation fp8 scale (t-streams ~ sqrt(2)x)

_CACHE = {}


def _build_bass():
    import concourse.bacc as bacc
    import concourse.bass as bass
    import concourse.tile as tile
    from concourse import mybir

    def bcast(ap2d, k):
        # Append a step-0 (broadcast) innermost dim to a 2D AP.
        return bass.AP(tensor=ap2d.tensor, offset=ap2d.offset,
                       ap=[*list(ap2d.ap), [0, k]])

    f32 = mybir.dt.float32
    bf16 = mybir.dt.bfloat16
    fp8 = mybir.dt.float8e4
    Alu = mybir.AluOpType
    DR = mybir.MatmulPerfMode.DoubleRow

    nc = bacc.Bacc(
        "TRN2", target_bir_lowering=False, debug=False, num_devices=N_CORES)
    # xq: X^T laid out [partition, col, di, hi/lo] — di and the hi/lo fp8
    # planes innermost.  Any DoubleRow slot pair (cross-di or hi/lo) then
    # spans a narrow byte interval, so the interval-based dependency
    # tracker ties each matmul only to its own column range's DMA piece;
    # pieces are >=4.8KB contiguous rows (no small-descriptor penalty).
    # Zero-padded halo col on each side.
    xq_d = nc.dram_tensor("xq", [128, (L + 2) * 2 * NT], fp8,
                          kind="ExternalInput")
    # wc: per dout-tile, slot-major: [slot][ktile][dout] with slot0 = Wl,
    # slot1 = Wh (so the hot slot1 plane can be DMA'd first).
    wc_d = nc.dram_tensor("wc", [NT, 128, (NCOR + NKT) * 128], fp8,
                          kind="ExternalInput")
    # Winograd t-streams: [partition, pair, stream, di, hi/lo] and the
    # transformed weights: [dtile, partition, lo/hi, stream, di, dout].
    xt_d = nc.dram_tensor("xt", [128, NWP * XTSTR], fp8,
                          kind="ExternalInput")
    wc2_d = nc.dram_tensor("wc2", [NT, 128, 2 * 4 * NT * 128], fp8,
                           kind="ExternalInput")
    # c4 holds [const/4 | -const/4]: the negated copy biases the Winograd
    # (m3|m4') eviction so the A-transform picks up exactly +const/4 on
    # both output streams (it cancels inside v' = m4'-m3).
    c4_d = nc.dram_tensor("c4", [128, 2 * NT], f32, kind="ExternalInput")
    out_d = nc.dram_tensor("out", [D, L], bf16, kind="ExternalOutput")

    # xq DMA piece boundaries: one chunk + conv halo per piece (chunk 0
    # split in half so the first matmuls can start sooner).
    xcuts = [0, 206, 410, 818, 1226, 1634, L + 2]
    N_WARM = 34            # PE warm-up matmuls (p-state ramp cover)

    with tile.TileContext(nc) as tc:
        with (
            tc.tile_pool(name="const", bufs=1) as cpool,
            tc.tile_pool(name="ztmp", bufs=6) as zpool,
            tc.tile_pool(name="ptmp", bufs=6) as tpool,
            tc.tile_pool(name="psum", bufs=1, space="PSUM") as ppool,
        ):
            xq = cpool.tile([128, L + 2, NT, 2], fp8, tag="xq")
            wc = cpool.tile([128, NT, NCOR + NKT, 128], fp8, tag="wc")
            xt = cpool.tile([128, NWP, 4, NT, 2], fp8, tag="xt")
            wc2 = cpool.tile([128, NT, 2, 4, NT, 128], fp8, tag="wc2")
            c4 = cpool.tile([128, 2 * NT], f32, tag="c4")
            zb = cpool.tile([128, NT, L], bf16, tag="zb")

            wcv = wc_d.rearrange("t p (k m) -> t p k m", m=128)
            CSTR = 2 * NT  # bytes per column in xq (di x hi/lo)

            def xq_piece(c0, c1):
                nc.sync.dma_start(out=xq[:, c0:c1, :, :],
                                  in_=xq_d[:, CSTR * c0:CSTR * c1])

            # PE warm-up: matmuls on a memset scratch keep the tensor engine
            # continuously busy from t~0 so the p-state ramp completes while
            # the startup DMAs are in flight (PE dispatch is by readiness, so
            # real matmuls seamlessly take over as their data lands).
            # The scratch is tiny (memset is on the warm-up critical path);
            # the rhs broadcasts one column via a step-0 AP to keep the
            # 128-col matmul duration.
            warm = cpool.tile([128, 8], bf16, tag="warm")
            wps = ppool.tile([128, 128], f32, name="wps", tag="wps")
            nc.gpsimd.memset(warm, 0.0)
            wrhs = bass.AP(tensor=warm.tensor, offset=warm.offset,
                           ap=[[warm.ap[0][0], 128], [0, 128]])

            def warm_fill(n):
                # A burst of n throwaway matmuls: fills an expected PE stall
                # so the busy streak (and with it the p-state ramp) survives.
                for i in range(n):
                    nc.tensor.matmul(wps[0:8, :], warm, wrhs,
                                     start=(i == 0), stop=(i == n - 1))

            warm_fill(N_WARM)

            # DMA emission order = priority. The head is latency-bound
            # (HWDGE + DGE delay + 900ns completion-sem per piece), so the
            # first pieces are exactly what the first half-width units need:
            # xq cols 0..206, then the hot (Wh) weight halves of dt0/dt1,
            # then the rest in consumption order.
            wc2v = wc2_d.rearrange("t p (pl s di m) -> t p pl s di m",
                                   pl=2, s=4, di=NT, m=128)

            def xt_piece(p0, p1):
                nc.sync.dma_start(out=xt[:, p0:p1, :, :, :],
                                  in_=xt_d[:, XTSTR * p0:XTSTR * p1])

            # Winograd replaces the direct path for chunks 2-3, so xq
            # pieces [818:1634] are dropped; chunk 4's conv halo still
            # needs xq cols 1632/1633.  The W inputs (xt, wc2) are large
            # and late-deadline, so they go after the direct-head pieces,
            # interleaved in unit-consumption order.
            xq_piece(xcuts[0], xcuts[1])
            nc.sync.dma_start(out=wc[:, 0, NCOR:], in_=wcv[0][:, NCOR:])
            nc.sync.dma_start(out=wc[:, 1, NCOR:], in_=wcv[1][:, NCOR:])
            xq_piece(xcuts[1], xcuts[2])
            nc.sync.dma_start(out=wc[:, 0, 0:NCOR], in_=wcv[0][:, 0:NCOR])
            nc.sync.dma_start(out=wc[:, 1, 0:NCOR], in_=wcv[1][:, 0:NCOR])
            nc.sync.dma_start(out=c4[:, :], in_=c4_d[:, :])
            xq_piece(xcuts[2], xcuts[3])       # 410:818 (chunk 1 + halo)
            xq_piece(1632, L + 2)              # chunk 4 + halo
            nc.sync.dma_start(out=wc2[:, 0], in_=wc2v[0])
            xt_piece(0, 102)
            xt_piece(102, 204)
            nc.sync.dma_start(out=wc2[:, 1], in_=wc2v[1])
            nc.sync.dma_start(out=wc[:, 2], in_=wcv[2])
            nc.sync.dma_start(out=wc[:, 3], in_=wcv[3])
            xt_piece(204, 306)
            xt_piece(306, 408)
            nc.sync.dma_start(out=wc2[:, 2], in_=wc2v[2])
            nc.sync.dma_start(out=wc2[:, 3], in_=wc2v[3])
            nc.sync.dma_start(out=wc[:, 4], in_=wcv[4])
            nc.sync.dma_start(out=wc[:, 5], in_=wcv[5])
            # dtiles 4-5 stay direct for chunks 2-3 (cols 1632..1633 are
            # already covered by the chunk-4 piece above — don't rewrite
            # them, a second writer would make the early chunk-4 units
            # wait on this late piece)
            xq_piece(xcuts[3], xcuts[4])
            xq_piece(xcuts[4], 1632)

            pstr = xq.ap[0][0]  # partition stride

            def xq_off(pl, di, col):
                return xq.offset + col * CSTR + di * 2 + pl

            # Units: (dt, base_col, width). The first chunk pair runs as
            # four half-width units with the main passes emitted before any
            # corrections (mains need only xq cols 0..206/410 + the hot
            # weight halves, so PE starts ~2.5us earlier); the end of the
            # stream is ordered so the post-PE tail is one unit's chain.
            # (dt, base, w, kind): globally ordered so every unit's inputs
            # land (DMA) just ahead of its matmuls, W-units sit away from
            # the head and the tail, and the stream ends on a half-width
            # direct unit (short post-PE chain).
            units = [(0, 0, 204, "d"), (1, 0, 204, "d"),
                     (0, 204, 204, "d"), (1, 204, 204, "d"),
                     (0, 1 * CHW, CHW, "d"), (1, 1 * CHW, CHW, "d"),
                     (0, 4 * CHW, CHW, "d"), (1, 4 * CHW, CHW, "d"),
                     (0, 2 * CHW, CHW, "W"), (1, 2 * CHW, CHW, "W"),
                     (2, 0 * CHW, CHW, "d"), (3, 0 * CHW, CHW, "d"),
                     (2, 1 * CHW, CHW, "d"), (3, 1 * CHW, CHW, "d"),
                     (0, 3 * CHW, CHW, "W"), (1, 3 * CHW, CHW, "W"),
                     (2, 2 * CHW, CHW, "W"), (3, 2 * CHW, CHW, "W"),
                     (2, 3 * CHW, CHW, "W"), (3, 3 * CHW, CHW, "W"),
                     (4, 0 * CHW, CHW, "d"), (5, 0 * CHW, CHW, "d"),
                     (4, 2 * CHW, CHW, "d"), (5, 2 * CHW, CHW, "d"),
                     (4, 1 * CHW, CHW, "d"), (5, 1 * CHW, CHW, "d"),
                     (4, 3 * CHW, CHW, "d"), (5, 3 * CHW, CHW, "d"),
                     (2, 4 * CHW, CHW, "d"), (3, 4 * CHW, CHW, "d"),
                     (4, 4 * CHW, CHW, "d"),
                     (5, 4 * CHW, 204, "d"), (5, 4 * CHW + 204, 204, "d")]
            n_units = len(units)

            sched = [("m", 0), ("w", 6), ("m", 1), ("w", 6),
                     ("m", 2), ("m", 3),
                     ("c", 0), ("p", 0), ("c", 1), ("p", 1),
                     ("c", 2), ("p", 2), ("c", 3), ("p", 3)]
            for i in range(4, n_units):
                if units[i][3] == "W":
                    sched += [("W", i)]
                else:
                    sched += [("m", i), ("c", i), ("p", i)]

            pscs = {}
            psn = [-1]  # psum buffer-name counter

            def next_ps():
                psn[0] += 1
                return f"ps{psn[0] % 7}"

            def emit_mains(ui):
                    dt, base, w, _ = units[ui]
                    nm = next_ps()
                    psc = ppool.tile([128, CHW], f32, name=nm,
                                     tag=nm)[:, 0:w]
                    pscs[ui] = psc
                    first = True
                    if True:
                        # main pass: slot pair = k-tiles (2t, 2t+1) of Wh
                        for t in range(NKT // 2):
                            di0, k0 = divmod(2 * t, 3)
                            di1, k1 = divmod(2 * t + 1, 3)
                            delta = (k1 - k0) * CSTR + (di1 - di0) * 2
                            rhs = bass.AP(
                                tensor=xq.tensor,
                                offset=xq_off(0, di0, base + k0),
                                ap=[[pstr, 128], [delta, 2], [CSTR, w]])
                            nc.tensor.matmul(
                                psc,
                                wc[:, dt, NCOR + 2 * t:NCOR + 2 * t + 2, :],
                                rhs,
                                start=first, stop=False, perf_mode=DR)
                            first = False

            def emit_corrs(ui):
                    dt, base, w, _ = units[ui]
                    psc = pscs[ui]
                    if True:
                        # full correction pass: slots (Wl, Xh), (Wh, Xl)
                        # for k-tiles 0..NCOR-1
                        for t in range(NCOR):
                            di, k = divmod(t, 3)
                            rhs = bass.AP(
                                tensor=xq.tensor,
                                offset=xq_off(0, di, base + k),
                                ap=[[pstr, 128], [1, 2], [CSTR, w]])
                            lhs = bass.AP(
                                tensor=wc.tensor,
                                offset=wc.offset
                                + (dt * (NCOR + NKT) + t) * 128,
                                ap=[[wc.ap[0][0], 128],
                                    [NCOR * 128, 2], [1, 128]])
                            nc.tensor.matmul(
                                psc,
                                lhs,
                                rhs,
                                start=False, stop=False,
                                perf_mode=DR)
                        # half corrections for the remaining tiles: X-side
                        # slot only (Xl @ Wh — drops the smaller W-side
                        # residual), paired like the main pass on the lo plane
                        for t in range(NCOR // 2, NKT // 2):
                            di0, k0 = divmod(2 * t, 3)
                            di1, k1 = divmod(2 * t + 1, 3)
                            delta = (k1 - k0) * CSTR + (di1 - di0) * 2
                            rhs = bass.AP(
                                tensor=xq.tensor,
                                offset=xq_off(1, di0, base + k0),
                                ap=[[pstr, 128], [delta, 2], [CSTR, w]])
                            nc.tensor.matmul(
                                psc,
                                wc[:, dt, NCOR + 2 * t:NCOR + 2 * t + 2, :],
                                rhs,
                                start=False,
                                stop=(t == NKT // 2 - 1),
                                perf_mode=DR)

            def emit_pool(ui):
                    dt, base, w, _ = units[ui]
                    psc = pscs[ui]
                    zh = zpool.tile([128, CHW], f32, name="zh",
                                    tag="zh")[:, 0:w]
                    nc.scalar.activation(
                        out=zh,
                        in_=psc,
                        func=mybir.ActivationFunctionType.Identity,
                        bias=c4[:, dt:dt + 1],
                        scale=1.0 / (SW * SX),
                    )
                    # late direct stretch: sum2 goes to the idle Pool engine
                    # to relieve the DVE backlog that otherwise drains as
                    # post-PE tail; the last units stay all-DVE (a Pool hop
                    # would sit on the final critical chain)
                    s2eng = (nc.gpsimd if 4 <= ui < n_units - 1
                             else nc.vector)
                    emit_poolcore(dt, base, w, zh, s2eng)

            def emit_poolcore(dt, base, w, zh, s2eng):
                    # ---- multi-scale pooling, chunk-local (w % 12 == 0) ----
                    z2 = zh.rearrange("p (n two) -> p n two", two=2)
                    z3 = zh.rearrange("p (n three) -> p n three", three=3)
                    z4 = zh.rearrange("p (n four) -> p n four", four=4)
                    zb3 = zb[:, dt, base:base + w].rearrange(
                        "p (n three) -> p n three", three=3)

                    sum2 = tpool.tile([128, CHW // 2], f32, name="sum2",
                                      tag="sum2")[:, 0:w // 2]
                    sum3 = tpool.tile([128, CHW // 3], f32, name="sum3",
                                      tag="sum3")[:, 0:w // 3]
                    wab = tpool.tile([128, CHW // 2], f32, name="wab",
                                     tag="wab")[:, 0:w // 2]
                    s2v = sum2.rearrange("p (n two) -> p n two", two=2)
                    wv2 = wab.rearrange("p (n two) -> p n two", two=2)

                    # sum2 on DVE for direct units (heads the W -> S2
                    # chain); on GPSIMD for Winograd units, whose DVE is
                    # loaded with the reconstruction ops. sum3 on GPSIMD
                    # in parallel.
                    s2eng.tensor_add(sum2, z2[:, :, 0], z2[:, :, 1])
                    nc.gpsimd.tensor_add(sum3, z3[:, :, 0], z3[:, :, 1])
                    nc.gpsimd.tensor_add(sum3, sum3, z3[:, :, 2])
                    # W / W' in ONE op: out[i,j] = 3*sum2[2i+j] + sum2[2i+1-j]
                    # via a swapped-pair (negative-step) view of sum2.
                    swp = bass.AP(tensor=sum2.tensor,
                                  offset=s2v[:, :, 1].offset,
                                  ap=[*list(s2v[:, :, 1].ap), [-1, 2]])
                    nc.vector.scalar_tensor_tensor(
                        wv2, s2v, 3.0, swp, Alu.mult, Alu.add)
                    # S = Z + 0.25 * rep(W|W'): positions 4i+{0,1} get W[i],
                    # 4i+{2,3} get W'[i], in ONE op: in0 walks wab with
                    # strides [2,1,0] producing W,W,W',W' per 4-col group
                    # (saves an op + a ~200ns semaphore hop per unit).
                    wrep = bass.AP(tensor=wab.tensor, offset=wab.offset,
                                   ap=[[wab.ap[0][0], 128], [2, w // 4],
                                       [1, 2], [0, 2]])
                    zv4 = zh.rearrange("p (n a b) -> p n a b", a=2, b=2)
                    nc.vector.scalar_tensor_tensor(
                        zv4, wrep, 0.25, zv4, Alu.mult, Alu.add)
                    # S += rep3(sum3)/3 in one op, writing the bf16 output
                    # buffer (sum3 broadcast over triples).
                    nc.vector.scalar_tensor_tensor(
                        zb3, bcast(sum3, 3), 1.0 / 3.0, z3,
                        Alu.mult, Alu.add)

                    # ship each finished piece immediately: the final DMA
                    # covers only the last unit, so the post-compute tail
                    # is short
                    nc.sync.dma_start(
                        out=out_d[dt * 128:(dt + 1) * 128, base:base + w],
                        in_=zb[:, dt, base:base + w])

            pstr2 = xt.ap[0][0]

            def xt_off(pl, s, di, j):
                return xt.offset + j * XTSTR + s * 12 + di * 2 + pl

            def emit_w(ui):
                    dt, base, w, _ = units[ui]
                    npair = w // 2
                    j0 = base // 2 - WBASE_PAIR
                    ems = []
                    for streams in ((0, 1), (2, 3)):
                        nm = next_ps()
                        pt = ppool.tile([128, CHW], f32, name=nm, tag=nm)
                        for si, s in enumerate(streams):
                            plane = pt[:, si * npair:(si + 1) * npair]
                            # main pass: Gh slot-pairs over di (contraction
                            # 2x128 per DR matmul, 3 per 768-deep stream)
                            for dj in range(3):
                                rhs = bass.AP(
                                    tensor=xt.tensor,
                                    offset=xt_off(0, s, 2 * dj, j0),
                                    ap=[[pstr2, 128], [2, 2], [XTSTR, npair]])
                                nc.tensor.matmul(
                                    plane,
                                    wc2[:, dt, 1, s, 2 * dj:2 * dj + 2, :],
                                    rhs,
                                    start=(dj == 0), stop=False, perf_mode=DR)
                            # FULL corrections on every k-tile: slots
                            # (Gl, Th), (Gh, Tl) exactly as the direct path
                            for di in range(NT):
                                rhs = bass.AP(
                                    tensor=xt.tensor,
                                    offset=xt_off(0, s, di, j0),
                                    ap=[[pstr2, 128], [1, 2], [XTSTR, npair]])
                                lhs = bass.AP(
                                    tensor=wc2.tensor,
                                    offset=wc2.offset
                                    + ((dt * 2 + 0) * 4 + s) * NT * 128
                                    + di * 128,
                                    ap=[[wc2.ap[0][0], 128],
                                        [4 * NT * 128, 2], [1, 128]])
                                nc.tensor.matmul(
                                    plane, lhs, rhs,
                                    start=False, stop=(di == NT - 1),
                                    perf_mode=DR)
                        em = zpool.tile([128, CHW], bf16,
                                        name=f"em{len(ems)}",
                                        tag=f"em{len(ems)}")
                        # bias +B on (m1|m2), -B on (m3|m4'): both z
                        # streams then pick up exactly +B in the plain
                        # tensor-tensor A-transform below.
                        boff = dt if len(ems) == 0 else NT + dt
                        nc.scalar.activation(
                            out=em, in_=pt,
                            func=mybir.ActivationFunctionType.Identity,
                            bias=c4[:, boff:boff + 1],
                            scale=1.0 / (SW * SX2))
                        ems.append(em)
                    em12, em34 = ems

                    # A-transform + interleave (biases already in the em
                    # planes):  z_even = (m1+m2) + m3,  z_odd = (m4'-m3) + m2.
                    # The u/v halves run on DVE; the interleaving adds run
                    # on GPSIMD, keeping the Winograd unit's DVE load equal
                    # to a direct unit's.
                    ub = tpool.tile([128, CHW // 2], bf16, name="ub",
                                    tag="ub")[:, 0:npair]
                    vb = tpool.tile([128, CHW // 2], bf16, name="vb",
                                    tag="vb")[:, 0:npair]
                    nc.vector.tensor_add(ub, em12[:, 0:npair],
                                         em12[:, npair:2 * npair])
                    nc.vector.tensor_sub(vb, em34[:, npair:2 * npair],
                                         em34[:, 0:npair])
                    zh = zpool.tile([128, CHW], f32, name="wzh",
                                    tag="wzh")[:, 0:w]
                    zhv = zh.rearrange("p (n two) -> p n two", two=2)
                    nc.gpsimd.tensor_add(zhv[:, :, 0], ub, em34[:, 0:npair])
                    nc.gpsimd.tensor_add(zhv[:, :, 1], vb,
                                         em12[:, npair:2 * npair])
                    emit_poolcore(dt, base, w, zh, nc.vector)

            for kind, arg in sched:
                if kind == "m":
                    emit_mains(arg)
                elif kind == "c":
                    emit_corrs(arg)
                elif kind == "p":
                    emit_pool(arg)
                elif kind == "W":
                    emit_w(arg)
                else:
                    warm_fill(arg)

    nc.compile()
    return nc


def _get_nc():
    if "nc" not in _CACHE:
        _CACHE["nc"] = _build_bass()
    return _CACHE["nc"]


def _q8(a):
    return a.astype(E4M3)


def _prep_host(X, conv_w, conv_b, wd, bd):
    """Fold wd into conv weights; fp8 hi/lo split; per-core transposed X."""
    # Wk_eff[k] = conv_w[:,:,k].T @ wd   (fp64), scaled by SW, split hi/lo.
    wc = np.empty((NT, 128, NCOR + NKT, 128), dtype=E4M3)
    for k in range(3):
        we = (conv_w[:, :, k].T.astype(np.float64)
              @ wd.astype(np.float64)) * SW        # [din, dout]
        wh = _q8(we.astype(np.float32))
        wl = _q8((we - wh.astype(np.float64)).astype(np.float32))
        # k-tile t = di*3 + k holds din block di; slot0 = Wl, slot1 = Wh.
        wh4 = wh.reshape(NT, 128, NT, 128)   # [di, p, dt, m]
        wl4 = wl.reshape(NT, 128, NT, 128)
        for di in range(NT):
            t = di * 3 + k
            if t < NCOR:
                wc[:, :, t, :] = wl4[di].transpose(1, 0, 2)  # [dt, p, m]
            wc[:, :, NCOR + t, :] = wh4[di].transpose(1, 0, 2)
    wc = np.ascontiguousarray(wc.reshape(NT, 128, (NCOR + NKT) * 128))

    const = 4.0 * (conv_b.astype(np.float64) @ wd.astype(np.float64)) \
        + bd.astype(np.float64)
    c4 = (const / 4.0).astype(np.float32).reshape(NT, 128).T
    c4 = np.ascontiguousarray(np.concatenate([c4, -c4], axis=1))

    xqs = []
    for n in range(X.shape[0]):
        xt = np.zeros((D, L + 2), dtype=np.float32)
        xt[:, 1:L + 1] = X[n].T * SX
        xh = _q8(xt)
        xl = _q8(xt - xh.astype(np.float32))
        xq = np.empty((128, L + 2, NT, 2), dtype=E4M3)
        xq[:, :, :, 0] = xh.reshape(NT, 128, L + 2).transpose(1, 2, 0)
        xq[:, :, :, 1] = xl.reshape(NT, 128, L + 2).transpose(1, 2, 0)
        xqs.append(xq.reshape(128, (L + 2) * 2 * NT))
    return xqs, wc, c4


def _prep_host_w(X, conv_w, wd):
    """Winograd F(2,3) operands for chunks W_CHUNKS (all dtiles).

    Transformed weights G = [G0, (G0+G1+G2)/2, (G0-G1+G2)/2, -G2] (wd
    folded), hi/lo fp8 at scale SW; t-streams per pair j:
    [x_{2j-1}-x_{2j+1}, x_{2j}+x_{2j+1}, x_{2j+1}-x_{2j}, x_{2j}-x_{2j+2}],
    hi/lo fp8 at scale SX2.  Then z_even = m1+m2+m3, z_odd = m2-m3+m4'.
    """
    We = [conv_w[:, :, k].T.astype(np.float64) @ wd.astype(np.float64)
          for k in range(3)]
    G = [We[0], (We[0] + We[1] + We[2]) * 0.5,
         (We[0] - We[1] + We[2]) * 0.5, -We[2]]
    wc2 = np.empty((NT, 128, 2, 4, NT, 128), dtype=E4M3)
    for s in range(4):
        gs = G[s] * SW
        gh = _q8(gs.astype(np.float32))
        gl = _q8((gs - gh.astype(np.float64)).astype(np.float32))
        gh4 = gh.reshape(NT, 128, NT, 128)   # [di, p, dt, m]
        gl4 = gl.reshape(NT, 128, NT, 128)
        for di in range(NT):
            wc2[:, :, 0, s, di, :] = gl4[di].transpose(1, 0, 2)
            wc2[:, :, 1, s, di, :] = gh4[di].transpose(1, 0, 2)
    wc2 = np.ascontiguousarray(wc2.reshape(NT, 128, 2 * 4 * NT * 128))

    j = np.arange(WBASE_PAIR, WBASE_PAIR + NWP)
    xts = []
    for n in range(X.shape[0]):
        xT = X[n].T.astype(np.float32)   # [D, L]; all cols 2j-1..2j+2 in range
        a, b = xT[:, 2 * j - 1], xT[:, 2 * j]
        c, d = xT[:, 2 * j + 1], xT[:, 2 * j + 2]
        t = np.stack([a - c, b + c, c - b, b - d], axis=0) * SX2  # [4, D, NWP]
        th = _q8(t)
        tl = _q8(t - th.astype(np.float32))
        xt = np.empty((128, NWP, 4, NT, 2), dtype=E4M3)
        xt[:, :, :, :, 0] = th.reshape(4, NT, 128, NWP).transpose(2, 3, 0, 1)
        xt[:, :, :, :, 1] = tl.reshape(4, NT, 128, NWP).transpose(2, 3, 0, 1)
        xts.append(xt.reshape(128, NWP * XTSTR))
    return xts, wc2


def _get_runner():
    """Cached jitted SPMD executor (mirrors bass2jax.run_bass_via_pjrt)."""
    if "runner" in _CACHE:
        return _CACHE["runner"]

    import jax
    import jax.numpy as jnp  # noqa: F401
    from jax.experimental.shard_map import shard_map
    from jax.sharding import Mesh, PartitionSpec
    import concourse.mybir as mybir
    from concourse import bass2jax

    nc = _get_nc()
    bass2jax.install_neuronx_cc_hook()

    part_name = nc.partition_id_tensor.name if nc.partition_id_tensor else None
    in_names, out_names, out_avals = [], [], []
    for alloc in nc.m.functions[0].allocations:
        if not isinstance(alloc, mybir.MemoryLocationSet):
            continue
        name = alloc.memorylocations[0].name
        if alloc.kind == "ExternalInput":
            if name != part_name:
                in_names.append(name)
        elif alloc.kind == "ExternalOutput":
            out_names.append(name)
            out_avals.append(jax.core.ShapedArray(
                tuple(alloc.tensor_shape), mybir.dt.np(alloc.dtype)))
    n_params = len(in_names)
    all_names = tuple(
        in_names + out_names + ([part_name] if part_name else []))

    def _body(*args):
        operands = list(args)
        if part_name is not None:
            operands.append(bass2jax.partition_id_tensor())
        outs = bass2jax._bass_exec_p.bind(
            *operands,
            out_avals=tuple(out_avals),
            in_names=all_names,
            out_names=tuple(out_names),
            lowering_input_output_aliases=(),
            sim_require_finite=True,
            sim_require_nnan=True,
            nc=nc,
        )
        return tuple(outs)

    devices = jax.devices()[:N_CORES]
    mesh = Mesh(np.asarray(devices), ("core",))
    n_outs = len(out_names)
    sharded = jax.jit(
        shard_map(_body, mesh=mesh,
                  in_specs=(PartitionSpec("core"),) * (n_params + n_outs),
                  out_specs=(PartitionSpec("core"),) * n_outs,
                  check_rep=False),
        donate_argnums=tuple(range(n_params, n_params + n_outs)),
        keep_unused=True,
    )
    # Device-side zero buffers for the donated outputs (avoids shipping
    # N_CORES * MBs of zeros through the tunnel every call).
    from jax.sharding import NamedSharding
    make_zeros = [
        jax.jit(
            (lambda shape, dtype: (lambda: jnp.zeros(shape, dtype)))(
                (N_CORES * a.shape[0], *a.shape[1:]), a.dtype),
            out_shardings=NamedSharding(mesh, PartitionSpec("core")))
        for a in out_avals
    ]
    _CACHE["runner"] = (sharded, in_names, out_names, out_avals, make_zeros)
    return _CACHE["runner"]


def kernel(**inputs):
    X = np.asarray(inputs["X"], dtype=np.float32)
    conv_w = np.asarray(inputs["conv_w"], dtype=np.float32)
    conv_b = np.asarray(inputs["conv_b"], dtype=np.float32)
    wd = np.asarray(inputs["wd"], dtype=np.float32)
    bd = np.asarray(inputs["bd"], dtype=np.float32)

    xqs, wc_host, c4 = _prep_host(X, conv_w, conv_b, wd, bd)
    xts, wc2_host = _prep_host_w(X, conv_w, wd)

    res = None
    for attempt in range(3):
        try:
            sharded, in_names, out_names, out_avals, make_zeros = _get_runner()
            per_core = {"xq": xqs, "wc": [wc_host] * N_CORES,
                        "c4": [c4] * N_CORES,
                        "xt": xts, "wc2": [wc2_host] * N_CORES}
            concat_in = [np.concatenate(per_core[nm], axis=0)
                         for nm in in_names]
            concat_zeros = [mz() for mz in make_zeros]
            out_arrs = sharded(*concat_in, *concat_zeros)
            res = np.asarray(out_arrs[out_names.index("out")])
            break
        except Exception:
            # Transient device wedge (can be inherited from a previous
            # crashed process on the shared terminal). Reset the PJRT
            # client and rebuild the jitted runner, then retry.
            if attempt == 2:
                raise
            import time
            import jax
            import jax._src.xla_bridge as _xb
            time.sleep(5.0)
            _CACHE.pop("runner", None)
            try:
                jax.clear_caches()
                _xb._clear_backends()
            except Exception:
                pass
    res = res.reshape(N_CORES, D, L).astype(np.float32)

    out = np.empty((N_SAMPLES, L, D), dtype=np.float32)
    for n in range(N_SAMPLES):
        out[n] = res[n].T
    return out



# revision 76
# speedup vs baseline: 1.0020x; 1.0020x over previous
"""Trainium2 Bass kernel for nn_GBSTokenizer.

Math: the reference's route softmax is over a size-1 axis, so the route
probabilities are exactly 1.0 and the L x L calibration matmul collapses to a
scalar ~1 (verified |s-1| < 6e-8, output deviation < 6e-7 absmax-relative).
The computation therefore reduces to

    out = poolsum(conv1d(X, conv_w) + conv_b) @ wd + bd

where poolsum(z)[l] = z[l] + mean2[l//2] + mean3[l//3] + mean4[l//4]
(multi-scale block means, blocks of size 2/3/4). Since everything between
the conv and the final projection is linear, wd is folded into the conv
weights on the host:  Wk_eff = conv_w[:,:,k].T @ wd, so the device computes

    Z[dout, l] = sum_k  Wk_eff[k].T @ XT[:, l+k-1]
    S = poolsum_over_l(Z) + const                          (vector ops)
    out[l, :]  = S[:, l]                                   (host transpose)

with const = 4*(conv_b @ wd) + bd added as a per-partition bias at PSUM
eviction (divided by 4 because poolsum multiplies constants by exactly 4).

Matmul precision: fp8(e4m3) DoubleRow with hi/lo error compensation.
Both W and X are split host-side into fp8 pairs at a shared power-of-2
scale (W*64 = Wh + Wl, X*16 = Xh + Xl, residuals stored at the same
scale so everything accumulates in one PSUM group):

    Z ~= (Xh@Wh + Xh@Wl + Xl@Wh) / 1024

Each DoubleRow matmul contracts 2 x 128 at 0.5 cycles/output column.
Corrections are asymmetric: 10 k-tiles get both slots, 8 k-tiles keep
only the X-side slot (Xl@Wh, paired like the main pass on the lo
plane), for 11.5 cyc/col vs 18 for bf16.

Chunks 2-3 of dtiles 0-3 instead use Winograd F(2,3) (see the W_CHUNKS
comment below): 12 k-tiles per output column with FULL corrections on
every tile = 9 cyc/col at ~4x lower quantization error than the direct
path.  More Winograd units don't pay: the A-transform adds DVE/Pool work
and those engines' effective time (busy + ~200ns/op semaphore hops)
already sits just under the PE window; past ~80% utilization the backlog
drains as post-PE tail.  Measured on HW: 1.892e-2 absmax-rel / 1.508e-2
rms-rel (gate 2e-2; absmax comes from the direct-path units).

Pooling combine per 408-col chunk (divisible by 12 = lcm(2,3,4), so all
blocks are chunk-local), with sum2[j] = pairsum, sum3[t] = triplesum:
    W [i] = 3*sum2[2i] +   sum2[2i+1]
    W'[i] =   sum2[2i] + 3*sum2[2i+1]
    S[4i+{0,1}] = Z + 0.25*W[i];  S[4i+{2,3}] = Z + 0.25*W'[i]
    S[3t+p]    += (1/3)*sum3[t]
The final combine writes bf16 (halves the output DMA).

Sharding: data-parallel over batch N=8, one sample per NeuronCore, params
replicated. All compute layouts keep feature dim on partitions and sequence
dim on the free axis (transposed), so the host transposes X in and out.
"""

import numpy as np
import ml_dtypes

# Problem shape (hardcoded per harness contract).
N_SAMPLES = 8
L = 2040
D = 768
NT = D // 128          # 6 partition tiles over features
NCH = 5                # l-chunks per psum pass
CHW = L // NCH         # 408 columns per chunk (<=512 fp32 = 1 PSUM bank),
                       # divisible by 12 so pooling is chunk-local
HALF = CHW             # full-chunk DoubleRow matmuls (moving free 816;
                       # the 512 limit is not enforced by walrus — validated
                       # numerically on HW)
NH = 1                 # matmul column-splits per chunk
NKT = 3 * NT           # 18 k-tiles (di-major: idx = di*3 + k)
NCOR = 10              # k-tiles with FULL hi/lo corrections; tiles
                       # NCOR..17 get the X-side correction slot only.
                       # HW measures 1.892e-2 absmax-rel vs the 2e-2 gate
                       # (above the host model's 1.778e-2 — the PE appears
                       # to flush fp8 subnormals on slot-0 operands, which
                       # weakens half-corrections; scaled-copy fixes cost
                       # more early DMA than they save in PE time)
N_CORES = 8

SW = 64.0              # weight fp8 scale
SX = 16.0              # activation fp8 scale
BF16 = ml_dtypes.bfloat16
E4M3 = ml_dtypes.float8_e4m3

# --- Winograd F(2,3) path (chunks 2..3) -----------------------------------
# For pair j (output cols 2j, 2j+1):
#   m1 = G0^T (x_{2j-1} - x_{2j+1});  m2 = Ga^T (x_{2j} + x_{2j+1})
#   m3 = Gb^T (x_{2j+1} - x_{2j});    m4'= G2n^T(x_{2j} - x_{2j+2})
# with Ga = (G0+G1+G2)/2, Gb = (G0-G1+G2)/2, G2n = -G2 folded on the host.
#   z_even = m1 + m2 + m3;  z_odd = m2 - m3 + m4'  (classic A-transform)
# 12 k-tiles of contraction per output column instead of 18, with FULL
# hi/lo corrections on every tile (0.75 cyc/ktile): 9 cyc/col vs the
# direct path's 11.5, at LOWER quantization error (~0.5% vs 1.9%, since
# nothing is half-corrected).  The 2x-finer W-unit error also absorbs the
# bf16 intermediates of the reconstruction.
W_CHUNKS = (2, 3)      # chunk indices computed via Winograd (all dtiles)
WBASE = W_CHUNKS[0] * CHW        # first Winograd output column
WBASE_PAIR = WBASE // 2          # first Winograd pair
NWP = len(W_CHUNKS) * CHW // 2   # Winograd pairs total
XTSTR = 4 * NT * 2               # bytes per pair-column in xt (s x di x pl)
SX2 = 32.0             # Winograd activation fp8 scale (t-streams ~ sqrt(2)x)

_CACHE = {}


def _build_bass():
    import concourse.bacc as bacc
    import concourse.bass as bass
    import concourse.tile as tile
    from concourse import mybir

    def bcast(ap2d, k):
        # Append a step-0 (broadcast) innermost dim to a 2D AP.
        return bass.AP(tensor=ap2d.tensor, offset=ap2d.offset,
                       ap=[*list(ap2d.ap), [0, k]])

    f32 = mybir.dt.float32
    bf16 = mybir.dt.bfloat16
    fp8 = mybir.dt.float8e4
    Alu = mybir.AluOpType
    DR = mybir.MatmulPerfMode.DoubleRow

    nc = bacc.Bacc(
        "TRN2", target_bir_lowering=False, debug=False, num_devices=N_CORES)
    # xq: X^T laid out [partition, col, di, hi/lo] — di and the hi/lo fp8
    # planes innermost.  Any DoubleRow slot pair (cross-di or hi/lo) then
    # spans a narrow byte interval, so the interval-based dependency
    # tracker ties each matmul only to its own column range's DMA piece;
    # pieces are >=4.8KB contiguous rows (no small-descriptor penalty).
    # Zero-padded halo col on each side.
    xq_d = nc.dram_tensor("xq", [128, (L + 2) * 2 * NT], fp8,
                          kind="ExternalInput")
    # wc: per dout-tile, slot-major: [slot][ktile][dout] with slot0 = Wl,
    # slot1 = Wh (so the hot slot1 plane can be DMA'd first).
    wc_d = nc.dram_tensor("wc", [NT, 128, (NCOR + NKT) * 128], fp8,
                          kind="ExternalInput")
    # Winograd t-streams: [partition, pair, stream, di, hi/lo] and the
    # transformed weights: [dtile, partition, lo/hi, stream, di, dout].
    xt_d = nc.dram_tensor("xt", [128, NWP * XTSTR], fp8,
                          kind="ExternalInput")
    wc2_d = nc.dram_tensor("wc2", [NT, 128, 2 * 4 * NT * 128], fp8,
                           kind="ExternalInput")
    # c4 holds [const/4 | -const/4]: the negated copy biases the Winograd
    # (m3|m4') eviction so the A-transform picks up exactly +const/4 on
    # both output streams (it cancels inside v' = m4'-m3).
    c4_d = nc.dram_tensor("c4", [128, 2 * NT], f32, kind="ExternalInput")
    out_d = nc.dram_tensor("out", [D, L], bf16, kind="ExternalOutput")

    # xq DMA piece boundaries: one chunk + conv halo per piece (chunk 0
    # split in half so the first matmuls can start sooner).
    xcuts = [0, 206, 410, 818, 1226, 1634, L + 2]
    N_WARM = 34            # PE warm-up matmuls (p-state ramp cover)

    with tile.TileContext(nc) as tc:
        with (
            tc.tile_pool(name="const", bufs=1) as cpool,
            tc.tile_pool(name="ztmp", bufs=6) as zpool,
            tc.tile_pool(name="ptmp", bufs=6) as tpool,
            tc.tile_pool(name="psum", bufs=1, space="PSUM") as ppool,
        ):
            xq = cpool.tile([128, L + 2, NT, 2], fp8, tag="xq")
            wc = cpool.tile([128, NT, NCOR + NKT, 128], fp8, tag="wc")
            xt = cpool.tile([128, NWP, 4, NT, 2], fp8, tag="xt")
            wc2 = cpool.tile([128, NT, 2, 4, NT, 128], fp8, tag="wc2")
            c4 = cpool.tile([128, 2 * NT], f32, tag="c4")
            zb = cpool.tile([128, NT, L], bf16, tag="zb")

            wcv = wc_d.rearrange("t p (k m) -> t p k m", m=128)
            CSTR = 2 * NT  # bytes per column in xq (di x hi/lo)

            def xq_piece(c0, c1):
                nc.sync.dma_start(out=xq[:, c0:c1, :, :],
                                  in_=xq_d[:, CSTR * c0:CSTR * c1])

            # PE warm-up: matmuls on a memset scratch keep the tensor engine
            # continuously busy from t~0 so the p-state ramp completes while
            # the startup DMAs are in flight (PE dispatch is by readiness, so
            # real matmuls seamlessly take over as their data lands).
            # The scratch is tiny (memset is on the warm-up critical path);
            # the rhs broadcasts one column via a step-0 AP to keep the
            # 128-col matmul duration.
            warm = cpool.tile([128, 8], bf16, tag="warm")
            wps = ppool.tile([128, 128], f32, name="wps", tag="wps")
            nc.gpsimd.memset(warm, 0.0)
            wrhs = bass.AP(tensor=warm.tensor, offset=warm.offset,
                           ap=[[warm.ap[0][0], 128], [0, 128]])

            def warm_fill(n):
                # A burst of n throwaway matmuls: fills an expected PE stall
                # so the busy streak (and with it the p-state ramp) survives.
                for i in range(n):
                    nc.tensor.matmul(wps[0:8, :], warm, wrhs,
                                     start=(i == 0), stop=(i == n - 1))

            warm_fill(N_WARM)

            # DMA emission order = priority. The head is latency-bound
            # (HWDGE + DGE delay + 900ns completion-sem per piece), so the
            # first pieces are exactly what the first half-width units need:
            # xq cols 0..206, then the hot (Wh) weight halves of dt0/dt1,
            # then the rest in consumption order.
            wc2v = wc2_d.rearrange("t p (pl s di m) -> t p pl s di m",
                                   pl=2, s=4, di=NT, m=128)

            def xt_piece(p0, p1):
                nc.sync.dma_start(out=xt[:, p0:p1, :, :, :],
                                  in_=xt_d[:, XTSTR * p0:XTSTR * p1])

            # Winograd replaces the direct path for chunks 2-3, so xq
            # pieces [818:1634] are dropped; chunk 4's conv halo still
            # needs xq cols 1632/1633.  The W inputs (xt, wc2) are large
            # and late-deadline, so they go after the direct-head pieces,
            # interleaved in unit-consumption order.
            xq_piece(xcuts[0], xcuts[1])
            nc.sync.dma_start(out=wc[:, 0, NCOR:], in_=wcv[0][:, NCOR:])
            nc.sync.dma_start(out=wc[:, 1, NCOR:], in_=wcv[1][:, NCOR:])
            xq_piece(xcuts[1], xcuts[2])
            nc.sync.dma_start(out=wc[:, 0, 0:NCOR], in_=wcv[0][:, 0:NCOR])
            nc.sync.dma_start(out=wc[:, 1, 0:NCOR], in_=wcv[1][:, 0:NCOR])
            nc.sync.dma_start(out=c4[:, :], in_=c4_d[:, :])
            xq_piece(xcuts[2], xcuts[3])       # 410:818 (chunk 1 + halo)
            xq_piece(1632, L + 2)              # chunk 4 + halo
            nc.sync.dma_start(out=wc2[:, 0], in_=wc2v[0])
            xt_piece(0, 102)
            xt_piece(102, 204)
            nc.sync.dma_start(out=wc2[:, 1], in_=wc2v[1])
            nc.sync.dma_start(out=wc[:, 2], in_=wcv[2])
            nc.sync.dma_start(out=wc[:, 3], in_=wcv[3])
            xt_piece(204, 306)
            xt_piece(306, 408)
            nc.sync.dma_start(out=wc2[:, 2], in_=wc2v[2])
            nc.sync.dma_start(out=wc2[:, 3], in_=wc2v[3])
            nc.sync.dma_start(out=wc[:, 4], in_=wcv[4])
            nc.sync.dma_start(out=wc[:, 5], in_=wcv[5])
            # dtiles 4-5 stay direct for chunks 2-3 (cols 1632..1633 are
            # already covered by the chunk-4 piece above — don't rewrite
            # them, a second writer would make the early chunk-4 units
            # wait on this late piece)
            xq_piece(xcuts[3], xcuts[4])
            xq_piece(xcuts[4], 1632)

            pstr = xq.ap[0][0]  # partition stride

            def xq_off(pl, di, col):
                return xq.offset + col * CSTR + di * 2 + pl

            # Units: (dt, base_col, width). The first chunk pair runs as
            # four half-width units with the main passes emitted before any
            # corrections (mains need only xq cols 0..206/410 + the hot
            # weight halves, so PE starts ~2.5us earlier); the end of the
            # stream is ordered so the post-PE tail is one unit's chain.
            # (dt, base, w, kind): globally ordered so every unit's inputs
            # land (DMA) just ahead of its matmuls, W-units sit away from
            # the head and the tail, and the stream ends on a half-width
            # direct unit (short post-PE chain).
            units = [(0, 0, 204, "d"), (1, 0, 204, "d"),
                     (0, 204, 204, "d"), (1, 204, 204, "d"),
                     (0, 1 * CHW, CHW, "d"), (1, 1 * CHW, CHW, "d"),
                     (0, 4 * CHW, CHW, "d"), (1, 4 * CHW, CHW, "d"),
                     (0, 2 * CHW, CHW, "W"), (1, 2 * CHW, CHW, "W"),
                     (2, 0 * CHW, CHW, "d"), (3, 0 * CHW, CHW, "d"),
                     (2, 1 * CHW, CHW, "d"), (3, 1 * CHW, CHW, "d"),
                     (0, 3 * CHW, CHW, "W"), (1, 3 * CHW, CHW, "W"),
                     (2, 2 * CHW, CHW, "W"), (3, 2 * CHW, CHW, "W"),
                     (2, 3 * CHW, CHW, "d"), (3, 3 * CHW, CHW, "d"),
                     (4, 0 * CHW, CHW, "d"), (5, 0 * CHW, CHW, "d"),
                     (4, 2 * CHW, CHW, "d"), (5, 2 * CHW, CHW, "d"),
                     (4, 1 * CHW, CHW, "d"), (5, 1 * CHW, CHW, "d"),
                     (4, 3 * CHW, CHW, "d"), (5, 3 * CHW, CHW, "d"),
                     (2, 4 * CHW, CHW, "d"), (3, 4 * CHW, CHW, "d"),
                     (4, 4 * CHW, CHW, "d"),
                     (5, 4 * CHW, 204, "d"), (5, 4 * CHW + 204, 204, "d")]
            n_units = len(units)

            sched = [("m", 0), ("w", 6), ("m", 1), ("w", 6),
                     ("m", 2), ("m", 3),
                     ("c", 0), ("p", 0), ("c", 1), ("p", 1),
                     ("c", 2), ("p", 2), ("c", 3), ("p", 3)]
            for i in range(4, n_units):
                if units[i][3] == "W":
                    sched += [("W", i)]
                else:
                    sched += [("m", i), ("c", i), ("p", i)]

            pscs = {}
            psn = [-1]  # psum buffer-name counter

            def next_ps():
                psn[0] += 1
                return f"ps{psn[0] % 7}"

            def emit_mains(ui):
                    dt, base, w, _ = units[ui]
                    nm = next_ps()
                    psc = ppool.tile([128, CHW], f32, name=nm,
                                     tag=nm)[:, 0:w]
                    pscs[ui] = psc
                    first = True
                    if True:
                        # main pass: slot pair = k-tiles (2t, 2t+1) of Wh
                        for t in range(NKT // 2):
                            di0, k0 = divmod(2 * t, 3)
                            di1, k1 = divmod(2 * t + 1, 3)
                            delta = (k1 - k0) * CSTR + (di1 - di0) * 2
                            rhs = bass.AP(
                                tensor=xq.tensor,
                                offset=xq_off(0, di0, base + k0),
                                ap=[[pstr, 128], [delta, 2], [CSTR, w]])
                            nc.tensor.matmul(
                                psc,
                                wc[:, dt, NCOR + 2 * t:NCOR + 2 * t + 2, :],
                                rhs,
                                start=first, stop=False, perf_mode=DR)
                            first = False

            def emit_corrs(ui):
                    dt, base, w, _ = units[ui]
                    psc = pscs[ui]
                    if True:
                        # full correction pass: slots (Wl, Xh), (Wh, Xl)
                        # for k-tiles 0..NCOR-1
                        for t in range(NCOR):
                            di, k = divmod(t, 3)
                            rhs = bass.AP(
                                tensor=xq.tensor,
                                offset=xq_off(0, di, base + k),
                                ap=[[pstr, 128], [1, 2], [CSTR, w]])
                            lhs = bass.AP(
                                tensor=wc.tensor,
                                offset=wc.offset
                                + (dt * (NCOR + NKT) + t) * 128,
                                ap=[[wc.ap[0][0], 128],
                                    [NCOR * 128, 2], [1, 128]])
                            nc.tensor.matmul(
                                psc,
                                lhs,
                                rhs,
                                start=False, stop=False,
                                perf_mode=DR)
                        # half corrections for the remaining tiles: X-side
                        # slot only (Xl @ Wh — drops the smaller W-side
                        # residual), paired like the main pass on the lo plane
                        for t in range(NCOR // 2, NKT // 2):
                            di0, k0 = divmod(2 * t, 3)
                            di1, k1 = divmod(2 * t + 1, 3)
                            delta = (k1 - k0) * CSTR + (di1 - di0) * 2
                            rhs = bass.AP(
                                tensor=xq.tensor,
                                offset=xq_off(1, di0, base + k0),
                                ap=[[pstr, 128], [delta, 2], [CSTR, w]])
                            nc.tensor.matmul(
                                psc,
                                wc[:, dt, NCOR + 2 * t:NCOR + 2 * t + 2, :],
                                rhs,
                                start=False,
                                stop=(t == NKT // 2 - 1),
                                perf_mode=DR)

            def emit_pool(ui):
                    dt, base, w, _ = units[ui]
                    psc = pscs[ui]
                    zh = zpool.tile([128, CHW], f32, name="zh",
                                    tag="zh")[:, 0:w]
                    nc.scalar.activation(
                        out=zh,
                        in_=psc,
                        func=mybir.ActivationFunctionType.Identity,
                        bias=c4[:, dt:dt + 1],
                        scale=1.0 / (SW * SX),
                    )
                    # late direct stretch: sum2 goes to the idle Pool engine
                    # to relieve the DVE backlog that otherwise drains as
                    # post-PE tail; the last units stay all-DVE (a Pool hop
                    # would sit on the final critical chain)
                    s2eng = (nc.gpsimd if 4 <= ui < n_units - 1
                             else nc.vector)
                    emit_poolcore(dt, base, w, zh, s2eng)

            def emit_poolcore(dt, base, w, zh, s2eng):
                    # ---- multi-scale pooling, chunk-local (w % 12 == 0) ----
                    z2 = zh.rearrange("p (n two) -> p n two", two=2)
                    z3 = zh.rearrange("p (n three) -> p n three", three=3)
                    z4 = zh.rearrange("p (n four) -> p n four", four=4)
                    zb3 = zb[:, dt, base:base + w].rearrange(
                        "p (n three) -> p n three", three=3)

                    sum2 = tpool.tile([128, CHW // 2], f32, name="sum2",
                                      tag="sum2")[:, 0:w // 2]
                    sum3 = tpool.tile([128, CHW // 3], f32, name="sum3",
                                      tag="sum3")[:, 0:w // 3]
                    wab = tpool.tile([128, CHW // 2], f32, name="wab",
                                     tag="wab")[:, 0:w // 2]
                    s2v = sum2.rearrange("p (n two) -> p n two", two=2)
                    wv2 = wab.rearrange("p (n two) -> p n two", two=2)

                    # sum2 on DVE for direct units (heads the W -> S2
                    # chain); on GPSIMD for Winograd units, whose DVE is
                    # loaded with the reconstruction ops. sum3 on GPSIMD
                    # in parallel.
                    s2eng.tensor_add(sum2, z2[:, :, 0], z2[:, :, 1])
                    nc.gpsimd.tensor_add(sum3, z3[:, :, 0], z3[:, :, 1])
                    nc.gpsimd.tensor_add(sum3, sum3, z3[:, :, 2])
                    # W / W' in ONE op: out[i,j] = 3*sum2[2i+j] + sum2[2i+1-j]
                    # via a swapped-pair (negative-step) view of sum2.
                    swp = bass.AP(tensor=sum2.tensor,
                                  offset=s2v[:, :, 1].offset,
                                  ap=[*list(s2v[:, :, 1].ap), [-1, 2]])
                    nc.vector.scalar_tensor_tensor(
                        wv2, s2v, 3.0, swp, Alu.mult, Alu.add)
                    # S = Z + 0.25 * rep(W|W'): positions 4i+{0,1} get W[i],
                    # 4i+{2,3} get W'[i], in ONE op: in0 walks wab with
                    # strides [2,1,0] producing W,W,W',W' per 4-col group
                    # (saves an op + a ~200ns semaphore hop per unit).
                    wrep = bass.AP(tensor=wab.tensor, offset=wab.offset,
                                   ap=[[wab.ap[0][0], 128], [2, w // 4],
                                       [1, 2], [0, 2]])
                    zv4 = zh.rearrange("p (n a b) -> p n a b", a=2, b=2)
                    nc.vector.scalar_tensor_tensor(
                        zv4, wrep, 0.25, zv4, Alu.mult, Alu.add)
                    # S += rep3(sum3)/3 in one op, writing the bf16 output
                    # buffer (sum3 broadcast over triples).
                    nc.vector.scalar_tensor_tensor(
                        zb3, bcast(sum3, 3), 1.0 / 3.0, z3,
                        Alu.mult, Alu.add)

                    # ship each finished piece immediately: the final DMA
                    # covers only the last unit, so the post-compute tail
                    # is short
                    nc.sync.dma_start(
                        out=out_d[dt * 128:(dt + 1) * 128, base:base + w],
                        in_=zb[:, dt, base:base + w])

            pstr2 = xt.ap[0][0]

            def xt_off(pl, s, di, j):
                return xt.offset + j * XTSTR + s * 12 + di * 2 + pl

            def emit_w(ui):
                    dt, base, w, _ = units[ui]
                    npair = w // 2
                    j0 = base // 2 - WBASE_PAIR
                    ems = []
                    for streams in ((0, 1), (2, 3)):
                        nm = next_ps()
                        pt = ppool.tile([128, CHW], f32, name=nm, tag=nm)
                        for si, s in enumerate(streams):
                            plane = pt[:, si * npair:(si + 1) * npair]
                            # main pass: Gh slot-pairs over di (contraction
                            # 2x128 per DR matmul, 3 per 768-deep stream)
                            for dj in range(3):
                                rhs = bass.AP(
                                    tensor=xt.tensor,
                                    offset=xt_off(0, s, 2 * dj, j0),
                                    ap=[[pstr2, 128], [2, 2], [XTSTR, npair]])
                                nc.tensor.matmul(
                                    plane,
                                    wc2[:, dt, 1, s, 2 * dj:2 * dj + 2, :],
                                    rhs,
                                    start=(dj == 0), stop=False, perf_mode=DR)
                            # FULL corrections — slots (Gl, Th), (Gh, Tl)
                            # exactly as the direct path — on every k-tile
                            # of the shared m2/m3 planes.  The m1/m4'
                            # planes (streams 0 and 3, each feeding only
                            # ONE output stream) spend some of the W-path's
                            # error margin: di 4-5 get the X-side half
                            # correction only (one paired DR), saving one
                            # matmul per plane while W-column error stays
                            # ~1.5% < the direct path's 1.89% absmax.
                            nfull = 4 if s in (0, 3) else NT
                            for di in range(nfull):
                                rhs = bass.AP(
                                    tensor=xt.tensor,
                                    offset=xt_off(0, s, di, j0),
                                    ap=[[pstr2, 128], [1, 2], [XTSTR, npair]])
                                lhs = bass.AP(
                                    tensor=wc2.tensor,
                                    offset=wc2.offset
                                    + ((dt * 2 + 0) * 4 + s) * NT * 128
                                    + di * 128,
                                    ap=[[wc2.ap[0][0], 128],
                                        [4 * NT * 128, 2], [1, 128]])
                                nc.tensor.matmul(
                                    plane, lhs, rhs,
                                    start=False, stop=(di == NT - 1),
                                    perf_mode=DR)
                            if nfull < NT:
                                # half correction: slots (Gh_4, Tl_4),
                                # (Gh_5, Tl_5) in one DR matmul
                                rhs = bass.AP(
                                    tensor=xt.tensor,
                                    offset=xt_off(1, s, 4, j0),
                                    ap=[[pstr2, 128], [2, 2], [XTSTR, npair]])
                                nc.tensor.matmul(
                                    plane,
                                    wc2[:, dt, 1, s, 4:6, :],
                                    rhs,
                                    start=False, stop=True,
                                    perf_mode=DR)
                        em = zpool.tile([128, CHW], bf16,
                                        name=f"em{len(ems)}",
                                        tag=f"em{len(ems)}")
                        # bias +B on (m1|m2), -B on (m3|m4'): both z
                        # streams then pick up exactly +B in the plain
                        # tensor-tensor A-transform below.
                        boff = dt if len(ems) == 0 else NT + dt
                        nc.scalar.activation(
                            out=em, in_=pt,
                            func=mybir.ActivationFunctionType.Identity,
                            bias=c4[:, boff:boff + 1],
                            scale=1.0 / (SW * SX2))
                        ems.append(em)
                    em12, em34 = ems

                    # A-transform + interleave (biases already in the em
                    # planes):  z_even = (m1+m2) + m3,  z_odd = (m4'-m3) + m2.
                    # The u/v halves run on DVE; the interleaving adds run
                    # on GPSIMD, keeping the Winograd unit's DVE load equal
                    # to a direct unit's.
                    ub = tpool.tile([128, CHW // 2], bf16, name="ub",
                                    tag="ub")[:, 0:npair]
                    vb = tpool.tile([128, CHW // 2], bf16, name="vb",
                                    tag="vb")[:, 0:npair]
                    nc.vector.tensor_add(ub, em12[:, 0:npair],
                                         em12[:, npair:2 * npair])
                    nc.vector.tensor_sub(vb, em34[:, npair:2 * npair],
                                         em34[:, 0:npair])
                    zh = zpool.tile([128, CHW], f32, name="wzh",
                                    tag="wzh")[:, 0:w]
                    zhv = zh.rearrange("p (n two) -> p n two", two=2)
                    nc.gpsimd.tensor_add(zhv[:, :, 0], ub, em34[:, 0:npair])
                    nc.gpsimd.tensor_add(zhv[:, :, 1], vb,
                                         em12[:, npair:2 * npair])
                    emit_poolcore(dt, base, w, zh, nc.vector)

            for kind, arg in sched:
                if kind == "m":
                    emit_mains(arg)
                elif kind == "c":
                    emit_corrs(arg)
                elif kind == "p":
                    emit_pool(arg)
                elif kind == "W":
                    emit_w(arg)
                else:
                    warm_fill(arg)

    nc.compile()
    return nc


def _get_nc():
    if "nc" not in _CACHE:
        _CACHE["nc"] = _build_bass()
    return _CACHE["nc"]


def _q8(a):
    return a.astype(E4M3)


def _prep_host(X, conv_w, conv_b, wd, bd):
    """Fold wd into conv weights; fp8 hi/lo split; per-core transposed X."""
    # Wk_eff[k] = conv_w[:,:,k].T @ wd   (fp64), scaled by SW, split hi/lo.
    wc = np.empty((NT, 128, NCOR + NKT, 128), dtype=E4M3)
    for k in range(3):
        we = (conv_w[:, :, k].T.astype(np.float64)
              @ wd.astype(np.float64)) * SW        # [din, dout]
        wh = _q8(we.astype(np.float32))
        wl = _q8((we - wh.astype(np.float64)).astype(np.float32))
        # k-tile t = di*3 + k holds din block di; slot0 = Wl, slot1 = Wh.
        wh4 = wh.reshape(NT, 128, NT, 128)   # [di, p, dt, m]
        wl4 = wl.reshape(NT, 128, NT, 128)
        for di in range(NT):
            t = di * 3 + k
            if t < NCOR:
                wc[:, :, t, :] = wl4[di].transpose(1, 0, 2)  # [dt, p, m]
            wc[:, :, NCOR + t, :] = wh4[di].transpose(1, 0, 2)
    wc = np.ascontiguousarray(wc.reshape(NT, 128, (NCOR + NKT) * 128))

    const = 4.0 * (conv_b.astype(np.float64) @ wd.astype(np.float64)) \
        + bd.astype(np.float64)
    c4 = (const / 4.0).astype(np.float32).reshape(NT, 128).T
    c4 = np.ascontiguousarray(np.concatenate([c4, -c4], axis=1))

    xqs = []
    for n in range(X.shape[0]):
        xt = np.zeros((D, L + 2), dtype=np.float32)
        xt[:, 1:L + 1] = X[n].T * SX
        xh = _q8(xt)
        xl = _q8(xt - xh.astype(np.float32))
        xq = np.empty((128, L + 2, NT, 2), dtype=E4M3)
        xq[:, :, :, 0] = xh.reshape(NT, 128, L + 2).transpose(1, 2, 0)
        xq[:, :, :, 1] = xl.reshape(NT, 128, L + 2).transpose(1, 2, 0)
        xqs.append(xq.reshape(128, (L + 2) * 2 * NT))
    return xqs, wc, c4


def _prep_host_w(X, conv_w, wd):
    """Winograd F(2,3) operands for chunks W_CHUNKS (all dtiles).

    Transformed weights G = [G0, (G0+G1+G2)/2, (G0-G1+G2)/2, -G2] (wd
    folded), hi/lo fp8 at scale SW; t-streams per pair j:
    [x_{2j-1}-x_{2j+1}, x_{2j}+x_{2j+1}, x_{2j+1}-x_{2j}, x_{2j}-x_{2j+2}],
    hi/lo fp8 at scale SX2.  Then z_even = m1+m2+m3, z_odd = m2-m3+m4'.
    """
    We = [conv_w[:, :, k].T.astype(np.float64) @ wd.astype(np.float64)
          for k in range(3)]
    G = [We[0], (We[0] + We[1] + We[2]) * 0.5,
         (We[0] - We[1] + We[2]) * 0.5, -We[2]]
    wc2 = np.empty((NT, 128, 2, 4, NT, 128), dtype=E4M3)
    for s in range(4):
        gs = G[s] * SW
        gh = _q8(gs.astype(np.float32))
        gl = _q8((gs - gh.astype(np.float64)).astype(np.float32))
        gh4 = gh.reshape(NT, 128, NT, 128)   # [di, p, dt, m]
        gl4 = gl.reshape(NT, 128, NT, 128)
        for di in range(NT):
            wc2[:, :, 0, s, di, :] = gl4[di].transpose(1, 0, 2)
            wc2[:, :, 1, s, di, :] = gh4[di].transpose(1, 0, 2)
    wc2 = np.ascontiguousarray(wc2.reshape(NT, 128, 2 * 4 * NT * 128))

    j = np.arange(WBASE_PAIR, WBASE_PAIR + NWP)
    xts = []
    for n in range(X.shape[0]):
        xT = X[n].T.astype(np.float32)   # [D, L]; all cols 2j-1..2j+2 in range
        a, b = xT[:, 2 * j - 1], xT[:, 2 * j]
        c, d = xT[:, 2 * j + 1], xT[:, 2 * j + 2]
        t = np.stack([a - c, b + c, c - b, b - d], axis=0) * SX2  # [4, D, NWP]
        th = _q8(t)
        tl = _q8(t - th.astype(np.float32))
        xt = np.empty((128, NWP, 4, NT, 2), dtype=E4M3)
        xt[:, :, :, :, 0] = th.reshape(4, NT, 128, NWP).transpose(2, 3, 0, 1)
        xt[:, :, :, :, 1] = tl.reshape(4, NT, 128, NWP).transpose(2, 3, 0, 1)
        xts.append(xt.reshape(128, NWP * XTSTR))
    return xts, wc2


def _get_runner():
    """Cached jitted SPMD executor (mirrors bass2jax.run_bass_via_pjrt)."""
    if "runner" in _CACHE:
        return _CACHE["runner"]

    import jax
    import jax.numpy as jnp  # noqa: F401
    from jax.experimental.shard_map import shard_map
    from jax.sharding import Mesh, PartitionSpec
    import concourse.mybir as mybir
    from concourse import bass2jax

    nc = _get_nc()
    bass2jax.install_neuronx_cc_hook()

    part_name = nc.partition_id_tensor.name if nc.partition_id_tensor else None
    in_names, out_names, out_avals = [], [], []
    for alloc in nc.m.functions[0].allocations:
        if not isinstance(alloc, mybir.MemoryLocationSet):
            continue
        name = alloc.memorylocations[0].name
        if alloc.kind == "ExternalInput":
            if name != part_name:
                in_names.append(name)
        elif alloc.kind == "ExternalOutput":
            out_names.append(name)
            out_avals.append(jax.core.ShapedArray(
                tuple(alloc.tensor_shape), mybir.dt.np(alloc.dtype)))
    n_params = len(in_names)
    all_names = tuple(
        in_names + out_names + ([part_name] if part_name else []))

    def _body(*args):
        operands = list(args)
        if part_name is not None:
            operands.append(bass2jax.partition_id_tensor())
        outs = bass2jax._bass_exec_p.bind(
            *operands,
            out_avals=tuple(out_avals),
            in_names=all_names,
            out_names=tuple(out_names),
            lowering_input_output_aliases=(),
            sim_require_finite=True,
            sim_require_nnan=True,
            nc=nc,
        )
        return tuple(outs)

    devices = jax.devices()[:N_CORES]
    mesh = Mesh(np.asarray(devices), ("core",))
    n_outs = len(out_names)
    sharded = jax.jit(
        shard_map(_body, mesh=mesh,
                  in_specs=(PartitionSpec("core"),) * (n_params + n_outs),
                  out_specs=(PartitionSpec("core"),) * n_outs,
                  check_rep=False),
        donate_argnums=tuple(range(n_params, n_params + n_outs)),
        keep_unused=True,
    )
    # Device-side zero buffers for the donated outputs (avoids shipping
    # N_CORES * MBs of zeros through the tunnel every call).
    from jax.sharding import NamedSharding
    make_zeros = [
        jax.jit(
            (lambda shape, dtype: (lambda: jnp.zeros(shape, dtype)))(
                (N_CORES * a.shape[0], *a.shape[1:]), a.dtype),
            out_shardings=NamedSharding(mesh, PartitionSpec("core")))
        for a in out_avals
    ]
    _CACHE["runner"] = (sharded, in_names, out_names, out_avals, make_zeros)
    return _CACHE["runner"]


def kernel(**inputs):
    X = np.asarray(inputs["X"], dtype=np.float32)
    conv_w = np.asarray(inputs["conv_w"], dtype=np.float32)
    conv_b = np.asarray(inputs["conv_b"], dtype=np.float32)
    wd = np.asarray(inputs["wd"], dtype=np.float32)
    bd = np.asarray(inputs["bd"], dtype=np.float32)

    xqs, wc_host, c4 = _prep_host(X, conv_w, conv_b, wd, bd)
    xts, wc2_host = _prep_host_w(X, conv_w, wd)

    res = None
    for attempt in range(3):
        try:
            sharded, in_names, out_names, out_avals, make_zeros = _get_runner()
            per_core = {"xq": xqs, "wc": [wc_host] * N_CORES,
                        "c4": [c4] * N_CORES,
                        "xt": xts, "wc2": [wc2_host] * N_CORES}
            concat_in = [np.concatenate(per_core[nm], axis=0)
                         for nm in in_names]
            concat_zeros = [mz() for mz in make_zeros]
            out_arrs = sharded(*concat_in, *concat_zeros)
            res = np.asarray(out_arrs[out_names.index("out")])
            break
        except Exception:
            # Transient device wedge (can be inherited from a previous
            # crashed process on the shared terminal). Reset the PJRT
            # client and rebuild the jitted runner, then retry.
            if attempt == 2:
                raise
            import time
            import jax
            import jax._src.xla_bridge as _xb
            time.sleep(5.0)
            _CACHE.pop("runner", None)
            try:
                jax.clear_caches()
                _xb._clear_backends()
            except Exception:
                pass
    res = res.reshape(N_CORES, D, L).astype(np.float32)

    out = np.empty((N_SAMPLES, L, D), dtype=np.float32)
    for n in range(N_SAMPLES):
        out[n] = res[n].T
    return out

